# revision 1
# baseline (speedup 1.0000x reference)
"""Multi-head attention Trainium2 kernel (8 NeuronCores, SPMD).

Sharding: core c handles batch b=c//2, query-row half r=c%2 (1024 q rows),
all 8 heads. K/V for the batch are recomputed on both cores of a pair.

Per-core pipeline (all on one NeuronCore):
  QT = (8*Wq^T) x  , KT = Wk^T x  (fp32r matmuls, ~tf32 precision)
  V  = x Wv^T       (bf16)
  per (head, 128-row q tile):
     S = QT_h^T KT_h          (fp32r, K=64)            -> PSUM fp32
     masked = S + maskbias; rowmax (fused DVE tensor_tensor_reduce)
     P = exp(masked - rowmax) (ACT, bf16 out) + rowsum Z
     PT = transpose(P)        (PE identity transpose)  -> SBUF bf16
  per head: OT = V_h^T PT accumulation (bf16), O/Z, then the reference's
  head-scrambled reshape is folded into 8 strided K=64 matmuls vs Wo^T.
Output rows are (head, t) blocks; host reassembles the full [B,S,E] tensor.
"""

import os
import numpy as np
import ml_dtypes

import concourse.bass as bass
import concourse.mybir as mybir
from concourse import bacc
from concourse.bass_utils import run_bass_kernel_spmd
from concourse.tile import TileContext
from concourse.masks import make_identity

F32 = mybir.dt.float32
F32R = mybir.dt.float32r
BF16 = mybir.dt.bfloat16
AF = mybir.ActivationFunctionType
ALU = mybir.AluOpType

B, S, E, H, DK = 4, 2048, 512, 8, 64
SQ = S // 2          # q rows per core
NEG = -1000000000.0
N_CORES = 8

BF = ml_dtypes.bfloat16


def build_nc():
    nc = bacc.Bacc(None, target_bir_lowering=False)

    xqT = nc.declare_dram_parameter("xqT", [E, SQ], F32R, isOutput=False)
    xkT = nc.declare_dram_parameter("xkT", [E, S], F32R, isOutput=False)
    xvT = nc.declare_dram_parameter("xvT", [E, S], BF16, isOutput=False)
    mb = nc.declare_dram_parameter("mb", [SQ, S], BF16, isOutput=False)
    wqT = nc.declare_dram_parameter("wqT", [E, E], F32R, isOutput=False)
    wkT = nc.declare_dram_parameter("wkT", [E, E], F32R, isOutput=False)
    wvT = nc.declare_dram_parameter("wvT", [E, E], BF16, isOutput=False)
    woT = nc.declare_dram_parameter("woT", [E, E], BF16, isOutput=False)
    bqt = nc.declare_dram_parameter("bqt", [128, 4], F32, isOutput=False)
    bkt = nc.declare_dram_parameter("bkt", [128, 4], F32, isOutput=False)
    bvr = nc.declare_dram_parameter("bvr", [128, E], F32, isOutput=False)
    bor = nc.declare_dram_parameter("bor", [128, E], F32, isOutput=False)
    out = nc.declare_dram_parameter("out", [SQ, E], F32, isOutput=True)

    NQT = SQ // 128    # 8 q tiles
    NKC = S // 128     # 16 k chunks
    NE = E // 128      # 4 embed chunks

    with TileContext(nc) as tc:
        with (
            tc.tile_pool(name="const", bufs=1) as constp,
            tc.tile_pool(name="wo_p", bufs=1) as wo_p,
            tc.tile_pool(name="qkv", bufs=1) as qkv,
            tc.tile_pool(name="mbp", bufs=1) as mbp,
            tc.tile_pool(name="spsum", bufs=2, space="PSUM") as spsum,
        ):
            id_bf = constp.tile([128, 128], BF16, tag="id_bf", name="id_bf")
            make_identity(nc, id_bf[:, :])

            wo8 = [wo_p.tile([64, E], BF16, tag=f"wo8_{j}", name=f"wo8_{j}")
                   for j in range(8)]
            for j in range(8):
                nc.sync.dma_start(out=wo8[j][:, :],
                                  in_=woT[64 * j:64 * j + 64, :])
            bq_sb = constp.tile([128, 4], F32, tag="bq", name="bq")
            bk_sb = constp.tile([128, 4], F32, tag="bk", name="bk")
            bv_sb = constp.tile([128, E], F32, tag="bv", name="bv")
            bo_sb = constp.tile([128, E], F32, tag="bo", name="bo")
            nc.sync.dma_start(out=bq_sb[:, :], in_=bqt[:, :])
            nc.sync.dma_start(out=bk_sb[:, :], in_=bkt[:, :])
            nc.sync.dma_start(out=bv_sb[:, :], in_=bvr[:, :])
            nc.sync.dma_start(out=bo_sb[:, :], in_=bor[:, :])

            qt_sb = [qkv.tile([128, SQ], F32R, tag=f"qt{m}", name=f"qt{m}")
                     for m in range(NE)]
            kt_sb = [qkv.tile([128, S], F32R, tag=f"kt{m}", name=f"kt{m}")
                     for m in range(NE)]
            v_sb = [qkv.tile([128, E], BF16, tag=f"v{kc}", name=f"v{kc}")
                    for kc in range(NKC)]

            # ---- projection phase (x inputs + qkv weights scoped) ----
            with (
                tc.tile_pool(name="xin", bufs=1) as xin,
                tc.tile_pool(name="wts", bufs=1) as wts,
            ):
                wq_sb = [wts.tile([128, E], F32R, tag=f"wq{c}", name=f"wq{c}")
                         for c in range(NE)]
                wk_sb = [wts.tile([128, E], F32R, tag=f"wk{c}", name=f"wk{c}")
                         for c in range(NE)]
                wv_sb = [wts.tile([128, E], BF16, tag=f"wv{c}", name=f"wv{c}")
                         for c in range(NE)]
                xq_sb = [xin.tile([128, SQ], F32R, tag=f"xq{c}", name=f"xq{c}")
                         for c in range(NE)]
                xk_sb = [xin.tile([128, S], F32R, tag=f"xk{c}", name=f"xk{c}")
                         for c in range(NE)]
                xv_sb = [xin.tile([128, S], BF16, tag=f"xv{c}", name=f"xv{c}")
                         for c in range(NE)]
                for c in range(NE):
                    nc.sync.dma_start(out=wq_sb[c][:, :],
                                      in_=wqT[128 * c:128 * c + 128, :])
                    nc.sync.dma_start(out=wk_sb[c][:, :],
                                      in_=wkT[128 * c:128 * c + 128, :])
                    nc.sync.dma_start(out=wv_sb[c][:, :],
                                      in_=wvT[128 * c:128 * c + 128, :])
                    nc.sync.dma_start(out=xq_sb[c][:, :],
                                      in_=xqT[128 * c:128 * c + 128, :])
                    nc.sync.dma_start(out=xk_sb[c][:, :],
                                      in_=xkT[128 * c:128 * c + 128, :])
                    nc.sync.dma_start(out=xv_sb[c][:, :],
                                      in_=xvT[128 * c:128 * c + 128, :])

                for m in range(NE):
                    for n2 in range(SQ // 1024):
                        ps = spsum.tile([128, 1024], F32, tag="spsum",
                                        name="ps_q")
                        for half in range(2):
                            sl = slice(1024 * n2 + 512 * half,
                                       1024 * n2 + 512 * half + 512)
                            for c in range(NE):
                                nc.tensor.matmul(
                                    ps[:, 512 * half:512 * half + 512],
                                    wq_sb[c][:, 128 * m:128 * m + 128],
                                    xq_sb[c][:, sl],
                                    start=(c == 0), stop=(c == NE - 1))
                        nc.vector.tensor_scalar(
                            out=qt_sb[m][:, 1024 * n2:1024 * n2 + 1024],
                            in0=ps[:, :],
                            scalar1=bq_sb[:, m:m + 1], scalar2=None,
                            op0=ALU.add)
                    for n2 in range(S // 1024):
                        ps = spsum.tile([128, 1024], F32, tag="spsum",
                                        name="ps_k")
                        for half in range(2):
                            sl = slice(1024 * n2 + 512 * half,
                                       1024 * n2 + 512 * half + 512)
                            for c in range(NE):
                                nc.tensor.matmul(
                                    ps[:, 512 * half:512 * half + 512],
                                    wk_sb[c][:, 128 * m:128 * m + 128],
                                    xk_sb[c][:, sl],
                                    start=(c == 0), stop=(c == NE - 1))
                        nc.vector.tensor_scalar(
                            out=kt_sb[m][:, 1024 * n2:1024 * n2 + 1024],
                            in0=ps[:, :],
                            scalar1=bk_sb[:, m:m + 1], scalar2=None,
                            op0=ALU.add)
                for kc2 in range(NKC // 2):
                    ps = spsum.tile([128, 1024], F32, tag="spsum", name="ps_v")
                    for half in range(2):
                        kc = 2 * kc2 + half
                        for c in range(NE):
                            nc.tensor.matmul(
                                ps[:, 512 * half:512 * half + 512],
                                xv_sb[c][:, 128 * kc:128 * kc + 128],
                                wv_sb[c][:, :],
                                start=(c == 0), stop=(c == NE - 1))
                    for half in range(2):
                        kc = 2 * kc2 + half
                        nc.vector.tensor_tensor(
                            out=v_sb[kc][:, :],
                            in0=ps[:, 512 * half:512 * half + 512],
                            in1=bv_sb[:, :],
                            op=ALU.add)

            _STAGE = int(os.environ.get("KSTAGE", "9"))
            if _STAGE <= 1:
                _dbg = qkv.tile([128, E], F32, tag="dbg", name="dbg")
                nc.vector.tensor_copy(_dbg[:, :], qt_sb[0][:, 0:E].bitcast(F32))
                nc.sync.dma_start(out=out[0:128, :], in_=_dbg[:, :])
                nc.compile._noop if False else None
            # ---- attention phase ----
            with (
                tc.tile_pool(name="work", bufs=2) as work,
                tc.tile_pool(name="ptb", bufs=1) as ptb,
                tc.tile_pool(name="stats", bufs=3) as stats,
                tc.tile_pool(name="ptps", bufs=2, space="PSUM") as ptps,
                tc.tile_pool(name="otps", bufs=1, space="PSUM") as otps,
            ):
                mb_sb = [mbp.tile([128, S], BF16, tag=f"mb{j}", name=f"mb{j}")
                         for j in range(NQT)]
                for j in range(NQT):
                    nc.sync.dma_start(out=mb_sb[j][:, :],
                                      in_=mb[128 * j:128 * j + 128, :])

                for h in range(H if _STAGE >= 9 else (0 if _STAGE <= 1 else 1)):
                    hm, hp = h // 2, 64 * (h % 2)
                    qst = work.tile([64, SQ], F32R, tag="qst", name="qst")
                    kst = work.tile([64, S], F32R, tag="kst", name="kst")
                    nc.vector.tensor_copy(qst[:, :],
                                          qt_sb[hm][hp:hp + 64, :])
                    nc.vector.tensor_copy(kst[:, :],
                                          kt_sb[hm][hp:hp + 64, :])
                    zc = stats.tile([128, 2 * NQT], F32, tag="zc", name="zc")
                    ptbig = ptb.tile([128, NKC * 1024], BF16, tag="ptbig",
                                     name="ptbig")
                    ptbig_r = ptbig.rearrange("p (c q) -> p c q", c=NKC)
                    for j in range(NQT):
                        qsl = qst[:, 128 * j:128 * j + 128]
                        mx = stats.tile([128, 2], F32, tag="mx", name="mx")
                        sps = []
                        for half in range(2):
                            sp = spsum.tile([128, 1024], F32, tag="spsum",
                                            name="sp")
                            sps.append(sp)
                            for q4 in range(2):
                                ksl = slice(1024 * half + 512 * q4,
                                            1024 * half + 512 * q4 + 512)
                                nc.tensor.matmul(
                                    sp[:, 512 * q4:512 * q4 + 512],
                                    qsl, kst[:, ksl],
                                    start=True, stop=False)
                                nc.tensor.matmul(
                                    sp[:, 512 * q4:512 * q4 + 512],
                                    id_bf[:, :], mb_sb[j][:, ksl],
                                    start=False, stop=True)
                            nc.vector.tensor_reduce(
                                out=mx[:, half:half + 1], in_=sp[:, :],
                                axis=mybir.AxisListType.X, op=ALU.max)
                        nmx = stats.tile([128, 1], F32, tag="nmx", name="nmx")
                        nc.vector.tensor_reduce(
                            out=nmx[:, :], in_=mx[:, :],
                            axis=mybir.AxisListType.X,
                            op=ALU.max, negate=True)
                        p_sb = work.tile([128, S], BF16, tag="p", name="p")
                        for half in range(2):
                            nc.scalar.activation(
                                out=p_sb[:, 1024 * half:1024 * half + 1024],
                                in_=sps[half][:, :], func=AF.Exp,
                                bias=nmx[:, 0:1], scale=1.0,
                                accum_out=zc[:, 2 * j + half:2 * j + half + 1])
                        zj = stats.tile([128, 1], F32, tag="zj", name="zj")
                        nc.vector.tensor_reduce(
                            out=zj[:, :], in_=zc[:, 2 * j:2 * j + 2],
                            axis=mybir.AxisListType.X, op=ALU.add)
                        rzj = stats.tile([128, 1], F32, tag="rzj", name="rzj")
                        nc.vector.reciprocal(rzj[:, :], zj[:, :])
                        nc.vector.tensor_scalar(
                            out=p_sb[:, :], in0=p_sb[:, :], scalar1=rzj[:, 0:1],
                            scalar2=None, op0=ALU.mult)
                        if _STAGE <= 2:
                            _dbgp = work.tile([128, E], F32, tag="osb", name="osb2")
                            nc.vector.tensor_copy(_dbgp[:, :], p_sb[:, 0:E])
                            nc.sync.dma_start(out=out[128*j:128*j+128, :], in_=_dbgp[:, :])
                            continue
                        for half in range(2):
                            pt_ps = ptps.tile([128, 1024], BF16, tag="ptps",
                                              name="pt_ps")
                            for c8 in range(8):
                                c = 8 * half + c8
                                nc.tensor.transpose(
                                    pt_ps[:, 128 * c8:128 * c8 + 128],
                                    p_sb[:, 128 * c:128 * c + 128],
                                    id_bf[:, :])
                            cp_out = ptbig_r[:, 8 * half:8 * half + 8,
                                             128 * j:128 * j + 128]
                            cp_in = pt_ps.rearrange(
                                "p (c q) -> p c q", c=8)[:, :, :]
                            if j % 2 == 0:
                                nc.vector.tensor_copy(cp_out, cp_in)
                            else:
                                nc.scalar.copy(out=cp_out, in_=cp_in)

                    if _STAGE <= 3:
                        continue
                    # PV: OT[d, q] accumulated over k chunks
                    ot = otps.tile([64, SQ], F32, tag="ot", name="ot")
                    for sh in range(SQ // 512):
                        for c in range(NKC):
                            nc.tensor.matmul(
                                ot[:, 512 * sh:512 * sh + 512],
                                v_sb[c][:, 64 * h:64 * h + 64],
                                ptbig_r[:, c, 512 * sh:512 * sh + 512],
                                start=(c == 0), stop=(c == NKC - 1))

                    otd = work.tile([64, SQ], BF16, tag="otd", name="otd")
                    nc.vector.tensor_copy(
                        otd.rearrange("p (j t) -> p j t", j=8)[:, :, :],
                        ot.rearrange("p (t j) -> p j t", j=8)[:, :, :])

                    # final projection with the head-scramble folded in
                    po = spsum.tile([128, 1024], F32, tag="spsum", name="po")
                    for jj in range(8):
                        nc.tensor.matmul(
                            po[:, 0:512],
                            otd[:, 128 * jj:128 * jj + 128],
                            wo8[jj][:, :],
                            start=(jj == 0), stop=(jj == 7))
                    o_sb = work.tile([128, E], F32, tag="osb", name="osb")
                    nc.vector.tensor_tensor(
                        out=o_sb[:, :], in0=po[:, 0:512],
                        in1=bo_sb[:, :], op=ALU.add)
                    nc.sync.dma_start(out=out[128 * h:128 * h + 128, :],
                                      in_=o_sb[:, :])

    nc.compile()
    return nc


_NC = None
_last_in_maps = None


def _get_nc():
    global _NC
    if _NC is None:
        _NC = build_nc()
    return _NC


def kernel(query, key_in, value, mask, Wq, bq, Wk, bk, Wv, bv, Wo, bo):
    query = np.asarray(query, np.float32)
    key_in = np.asarray(key_in, np.float32)
    value = np.asarray(value, np.float32)
    mask = np.asarray(mask)
    Wq = np.asarray(Wq, np.float32)
    Wk = np.asarray(Wk, np.float32)
    Wv = np.asarray(Wv, np.float32)
    Wo = np.asarray(Wo, np.float32)

    wqT = np.ascontiguousarray((Wq * np.sqrt(DK)).T)
    wkT = np.ascontiguousarray(Wk.T)
    wvT = np.ascontiguousarray(Wv.T).astype(BF)
    woT = np.ascontiguousarray(Wo.T).astype(BF)
    bqt = np.ascontiguousarray(
        (np.asarray(bq, np.float32) * np.sqrt(DK)).reshape(4, 128).T)
    bkt = np.ascontiguousarray(np.asarray(bk, np.float32).reshape(4, 128).T)
    bvr = np.ascontiguousarray(np.tile(np.asarray(bv, np.float32).reshape(1, E), (128, 1)))
    bor = np.ascontiguousarray(np.tile(np.asarray(bo, np.float32).reshape(1, E), (128, 1)))

    mbias = (mask.astype(np.float32) - 1.0) * -NEG  # 0 where mask=1 else NEG
    in_maps = []
    for c in range(N_CORES):
        b, r = c // 2, c % 2
        q0 = SQ * r
        in_maps.append({
            "xqT": np.ascontiguousarray(query[b, q0:q0 + SQ, :].T),
            "xkT": np.ascontiguousarray(key_in[b].T),
            "xvT": np.ascontiguousarray(value[b].T).astype(BF),
            "mb": mbias[b, q0:q0 + SQ, :].astype(BF),
            "wqT": wqT, "wkT": wkT, "wvT": wvT, "woT": woT,
            "bqt": bqt, "bkt": bkt, "bvr": bvr, "bor": bor,
        })

    nc = _get_nc()
    global _last_in_maps
    _last_in_maps = in_maps
    res = run_bass_kernel_spmd(nc, in_maps, list(range(N_CORES)))

    full = np.empty((B, S, E), np.float32)
    for c in range(N_CORES):
        b, r = c // 2, c % 2
        oc = res.results[c]["out"]
        for h in range(H):
            full[b, 256 * h + 128 * r:256 * h + 128 * r + 128, :] = \
                oc[128 * h:128 * h + 128, :]
    return full

